# revision 1
# baseline (speedup 1.0000x reference)
"""Trainium2 Bass kernel: Tacotron-style location-sensitive attention step.

Sharding strategy (8 NeuronCores, SPMD):
  - Attention / conv / softmax / context: pure batch parallelism
    (B=128 -> 16 examples per core; enc_seq, proc_mem, attention weights,
    mask sharded on the batch dim host-side).
  - LSTM cell: H-sharded. Core j computes h.T rows [128j, 128j+128) for the
    FULL batch using only 1/8 of W_ih/W_hh (3.5 MB instead of 28 MB of
    replicated weight traffic), then a small AllGather of h.T.
    qry2 = h @ W_q.T + const is computed for the full batch and each core
    selects its 16 rows with a one-hot matmul (bsel input) so the SPMD
    graph stays core-uniform.

Compute dtypes: bf16 operands for all large matmuls / elementwise (well
inside the 2e-2 rel-err budget), f32 for PSUM, softmax and reductions.

kernel(**inputs) takes FULL numpy inputs (as produced by setup_inputs())
and returns the FULL [128, 512] float32 context.
"""

import sys

sys.path.insert(0, "/opt/trn_rl_repo")

import numpy as np

import concourse.bass as bass
import concourse.mybir as mybir
from concourse import bacc
from concourse.bass_utils import run_bass_kernel_spmd
from concourse.masks import make_identity
from concourse.bass import _add_dep_helper
from concourse.tile import TileContext

F32 = mybir.dt.float32
BF16 = mybir.dt.bfloat16
AF = mybir.ActivationFunctionType

B, S, E, P, H, A, F, KW = 128, 1024, 512, 256, 1024, 128, 32, 31
NCORES = 8
BL = B // NCORES        # 16 examples per core
HL = H // NCORES        # 128 h rows per core
PE_DIM = P + E          # 768
NKI = PE_DIM // 128     # 6
NKH = H // 128          # 8
NC_S = S // 128         # 8 s-chunks
PADW = KW // 2          # 15
CONVROW = PADW + S + 17  # 1056 padded per-channel staging row
TAPS = 2 * KW           # 62
ENC_T = 8               # s-chunks per enc DMA tile (whole example)
ENC_BUFS = 10
GRP = 4                 # examples per fused softmax/context group


def build():
    nc = bacc.Bacc("TRN2", target_bir_lowering=False, debug=False,
                   num_devices=NCORES)

    dp = nc.declare_dram_parameter
    prenet = dp("prenet", [B, P], F32, isOutput=False)
    prev_ctx = dp("prev_ctx", [B, E], F32, isOutput=False)
    att_h = dp("att_h", [B, H], F32, isOutput=False)
    att_c_sh = dp("att_c_sh", [B, HL], F32, isOutput=False)
    w_ih_sh = dp("w_ih_sh", [4, HL, PE_DIM], F32, isOutput=False)
    w_hh_sh = dp("w_hh_sh", [4, HL, H], F32, isOutput=False)
    b_ih_sh = dp("b_ih_sh", [4, HL], F32, isOutput=False)
    b_hh_sh = dp("b_hh_sh", [4, HL], F32, isOutput=False)
    prev_w = dp("prev_w", [BL, S], F32, isOutput=False)
    cum_w = dp("cum_w", [BL, S], F32, isOutput=False)
    enc = dp("enc", [BL, S, E], F32, isOutput=False)
    proc = dp("proc", [BL, S, A], F32, isOutput=False)
    conv_w = dp("conv_w", [F, 2, KW], F32, isOutput=False)
    conv_b = dp("conv_b", [F, 1], F32, isOutput=False)
    w_loc = dp("w_loc", [A, F], F32, isOutput=False)
    b_loc = dp("b_loc", [1, A], F32, isOutput=False)
    w_q = dp("w_q", [A, H], F32, isOutput=False)
    b_q = dp("b_q", [1, A], F32, isOutput=False)
    w_out = dp("w_out", [1, A], F32, isOutput=False)
    bsel = dp("bsel", [B, BL], F32, isOutput=False)
    out = dp("out", [BL, E], F32, isOutput=True)

    with TileContext(nc) as tc:
        with (
            tc.tile_pool(name="const", bufs=1) as cpool,
            tc.tile_pool(name="work", bufs=2) as wpool,
            tc.tile_pool(name="xpadp", bufs=6) as xpool,
            tc.tile_pool(name="conv", bufs=16) as convpool,
            tc.tile_pool(name="proc", bufs=16) as ppool,
            tc.tile_pool(name="vbig", bufs=2) as vpool,
            tc.tile_pool(name="psA", bufs=2, space="PSUM") as psA,
            tc.tile_pool(name="psV", bufs=2, space="PSUM") as psV,
            tc.tile_pool(name="psX", bufs=2, space="PSUM") as psX,
            tc.tile_pool(name="dram", bufs=1, space="DRAM") as dpool,
        ):
            def mm_ps(shape):
                t = psA.tile([128, 512], F32, tag="mm")
                return t[: shape[0], : shape[1]]

            # ------------- constants / small preprocessing -------------
            ident = cpool.tile([128, 128], F32)
            make_identity(nc, ident[:])
            id_bf = cpool.tile([128, 128], BF16)
            nc.vector.tensor_copy(id_bf[:], ident[:])
            ones_row = cpool.tile([1, 128], F32)
            nc.vector.memset(ones_row[:], 1.0)

            pe_t_ctr = [0]

            def pe_t(dst, src_ap, rows, engine=None):
                """dst = src_ap([rows, cols]).T via TensorE (+copy/cast)."""
                ps = mm_ps((dst.shape[0], rows))
                nc.tensor.transpose(ps, src_ap, ident[:rows, :rows])
                if engine is None:
                    pe_t_ctr[0] += 1
                    engine = "dve" if pe_t_ctr[0] % 2 else "act"
                if engine == "dve":
                    nc.vector.tensor_copy(dst, ps)
                else:
                    nc.scalar.copy(dst, ps)

            # ---- LSTM weight shard: load FIRST (DMA priority), PE-T, DVE copy
            # Lives in its own pool, closed after the gates so the SBUF is
            # recycled for the enc prefetch pool.
            NK = NKI + NKH  # 14
            wtpool_cm = tc.tile_pool(name="wt", bufs=1)
            wtpool = wtpool_cm.__enter__()
            wT = wtpool.tile([128, 4, NK, HL], BF16)
            wnats = []
            for g in range(4):
                wi_nat = wtpool.tile([HL, PE_DIM], F32, tag=f"wload{g}")
                nc.sync.dma_start(wi_nat[:], w_ih_sh[g])
                wh_nat = wtpool.tile([HL, H], F32, tag=f"wload2{g}")
                nc.sync.dma_start(wh_nat[:], w_hh_sh[g])
                wnats.append((wi_nat, wh_nat))

            # activations for the LSTM (needed right after W)
            pn_nat = wtpool.tile([B, P], F32)
            nc.sync.dma_start(pn_nat[:], prenet[:])
            pc_nat = wtpool.tile([B, E], F32)
            nc.sync.dma_start(pc_nat[:], prev_ctx[:])
            ah_nat = wtpool.tile([B, H], F32)
            nc.sync.dma_start(ah_nat[:], att_h[:])
            ac_nat = wtpool.tile([B, HL], F32)
            nc.sync.dma_start(ac_nat[:], att_c_sh[:])
            bi_nat = wtpool.tile([4, HL], F32)
            nc.sync.dma_start(bi_nat[:], b_ih_sh[:])
            bh_nat = wtpool.tile([4, HL], F32)
            crit_last = nc.sync.dma_start(bh_nat[:], b_hh_sh[:])

            def pe_t_multi(dst_ap, srcs, rows, engine):
                """Transpose several 128-col chunks into one psA tile, then
                copy them out with a single wide copy."""
                ps = psA.tile([128, 512], F32, tag="mm")
                for i, s_ap in enumerate(srcs):
                    nc.tensor.transpose(ps[:, i * rows:(i + 1) * rows], s_ap,
                                        ident[:rows, :rows])
                if engine == "dve":
                    nc.vector.tensor_copy(dst_ap, ps[:, :len(srcs) * rows])
                else:
                    nc.scalar.copy(dst_ap, ps[:, :len(srcs) * rows])

            for g in range(4):
                wi_nat, wh_nat = wnats[g]
                chunks = [wi_nat[:, k * 128:(k + 1) * 128] for k in range(NKI)]
                chunks += [wh_nat[:, k * 128:(k + 1) * 128] for k in range(NKH)]
                NK2 = NKI + NKH
                for q in range(0, NK2, 4):
                    qs = chunks[q:q + 4]
                    pe_t_multi(wT[:, g, q:q + len(qs), :], qs, HL,
                               "dve" if (q // 4) % 2 else "act")

            inpT = cpool.tile([128, NKI, B], BF16)
            ichunks = [pn_nat[:, k * 128:(k + 1) * 128] for k in range(2)]
            ichunks += [pc_nat[:, k * 128:(k + 1) * 128] for k in range(4)]
            pe_t_multi(inpT[:, 0:4, :], ichunks[0:4], B, "act")
            pe_t_multi(inpT[:, 4:6, :], ichunks[4:6], B, "dve")
            ahT = cpool.tile([128, NKH, B], BF16)
            achunks = [ah_nat[:, k * 128:(k + 1) * 128] for k in range(NKH)]
            pe_t_multi(ahT[:, 0:4, :], achunks[0:4], B, "act")
            pe_t_multi(ahT[:, 4:8, :], achunks[4:8], B, "dve")
            acT = cpool.tile([HL, B], BF16)
            pe_t(acT[:], ac_nat[:], B)
            nc.vector.tensor_add(bi_nat[:], bi_nat[:], bh_nat[:])
            bias_sb = cpool.tile([HL, 4], F32)
            pe_t(bias_sb[:], bi_nat[:], 4)

            # ---- gates (H-shard, full batch) -> h.T shard, as early as possible
            gate_sb = []
            for g in range(4):
                ps = mm_ps((HL, B))
                for k in range(NKI):
                    nc.tensor.matmul(ps, wT[:, g, k, :], inpT[:, k, :],
                                     start=(k == 0), stop=False)
                for k in range(NKH):
                    nc.tensor.matmul(ps, wT[:, g, NKI + k, :], ahT[:, k, :],
                                     start=False, stop=(k == NKH - 1))
                sb = cpool.tile([HL, B], BF16, tag=f"gate{g}")
                fn = AF.Tanh if g == 2 else AF.Sigmoid
                nc.scalar.activation(sb[:], ps, fn, bias=bias_sb[:, g:g + 1])
                gate_sb.append(sb)

            cT = cpool.tile([HL, B], BF16)
            nc.vector.tensor_mul(cT[:], gate_sb[1][:], acT[:])
            tg = cpool.tile([HL, B], BF16)
            nc.vector.tensor_mul(tg[:], gate_sb[0][:], gate_sb[2][:])
            nc.vector.tensor_add(cT[:], cT[:], tg[:])
            nc.scalar.activation(tg[:], cT[:], AF.Tanh)
            hT_sh = cpool.tile([HL, B], BF16)
            nc.vector.tensor_mul(hT_sh[:], gate_sb[3][:], tg[:])
            h_in = dpool.tile([HL, B], BF16)
            nc.scalar.dma_start(h_in[:], hT_sh[:])
            wtpool_cm.__exit__(None, None, None)
            epool_cm = tc.tile_pool(name="enc", bufs=ENC_BUFS)
            epool = epool_cm.__enter__()

            # ---- small constant preprocessing (off the critical path)
            cw_nat = cpool.tile([F, TAPS], F32)
            nc.sync.dma_start(cw_nat[:], conv_w.rearrange("f c k -> f (c k)"))
            w2 = cpool.tile([TAPS, F], BF16)
            pe_t(w2[:], cw_nat[:], F)

            wl_nat = cpool.tile([A, F], F32)
            nc.sync.dma_start(wl_nat[:], w_loc[:])
            wlocT = cpool.tile([F, A], F32)
            pe_t(wlocT[:], wl_nat[:], A)

            cb_col = cpool.tile([F, 1], F32)
            nc.sync.dma_start(cb_col[:], conv_b[:])
            bl_row = cpool.tile([1, A], F32)
            nc.sync.dma_start(bl_row[:], b_loc[:])
            bq_row = cpool.tile([1, A], F32)
            nc.sync.dma_start(bq_row[:], b_q[:])
            ps = mm_ps((1, A))
            nc.tensor.matmul(ps, cb_col[:], wlocT[:], start=True, stop=True)
            const_row = cpool.tile([1, A], F32)
            nc.vector.tensor_add(const_row[:], ps, bl_row[:])
            nc.vector.tensor_add(const_row[:], const_row[:], bq_row[:])

            wo_row = cpool.tile([1, A], F32)
            nc.sync.dma_start(wo_row[:], w_out[:])
            ps = mm_ps((128, A))
            nc.tensor.matmul(ps, ones_row[:], wo_row[:], start=True, stop=True)
            wo_rep8 = cpool.tile([128, NC_S, A], BF16)
            for c in range(NC_S):
                nc.scalar.copy(wo_rep8[:, c, :], ps)

            sel_sb = cpool.tile([B, BL], F32)
            nc.sync.dma_start(sel_sb[:], bsel[:])


            wq_nat = cpool.tile([A, H], F32)
            nc.sync.dma_start(wq_nat[:], w_q[:])
            wqT = cpool.tile([128, NKH, A], BF16)
            qchunks = [wq_nat[:, k * 128:(k + 1) * 128] for k in range(NKH)]
            pe_t_multi(wqT[:, 0:4, :], qchunks[0:4], A, "act")
            pe_t_multi(wqT[:, 4:8, :], qchunks[4:8], A, "dve")

            # padded conv input rows staged to DRAM (bf16):
            # row layout per (b, c): [15 zeros | 1024 data | 17 zeros]
            stage = cpool.tile([BL, 2 * CONVROW], BF16)
            nc.vector.memset(stage[:], 0.0)
            nc.gpsimd.dma_start(stage[:, PADW:PADW + S], cum_w[:])
            nc.gpsimd.dma_start(stage[:, CONVROW + PADW:CONVROW + PADW + S],
                                prev_w[:])
            pad_dram = dpool.tile([BL, 2 * CONVROW], BF16)
            nc.sync.dma_start(pad_dram[:], stage[:])
            # materialize all 62 overlapping window rows per example in DRAM
            win_dram = dpool.tile([BL, TAPS, S], BF16)
            for c in range(2):
                sb2 = pad_dram[0, c * CONVROW:c * CONVROW + 1]
                wsrc = bass.AP(
                    tensor=sb2.tensor,
                    offset=sb2.offset,
                    ap=[[2 * CONVROW, BL], [1, KW], [1, S]],
                )
                db2 = win_dram[0, c * KW:c * KW + 1, 0:1]
                wdst = bass.AP(
                    tensor=db2.tensor,
                    offset=db2.offset,
                    ap=[[TAPS * S, BL], [S, KW], [1, S]],
                )
                nc.sync.dma_start(wdst, wsrc)

            # ---- streaming preloads (bf16 casts on the gpsimd queue)
            proc_tiles = []
            for b in range(6):
                pt = ppool.tile([128, NC_S, A], BF16, tag="proc")
                pdma = nc.gpsimd.dma_start(
                    pt[:], proc[b].rearrange("(p r) a -> p r a", r=NC_S))
                if b == 0:
                    _add_dep_helper(pdma.ins, crit_last.ins, sync=True,
                                    reason="preloads yield DMA BW to LSTM-critical loads")
                proc_tiles.append(pt)
            # ---- location conv (contiguous per-example window loads)
            conv_tiles = []
            for b in range(BL):
                xpadT = xpool.tile([TAPS, S], BF16, tag="xpad")
                nc.sync.dma_start(xpadT[:], win_dram[b])
                conv_sb = convpool.tile([F + 1, S], BF16, tag="conv")
                for h2 in range(2):
                    ps = mm_ps((F, 512))
                    nc.tensor.matmul(ps, w2[:],
                                     xpadT[:, h2 * 512:(h2 + 1) * 512],
                                     start=True, stop=True)
                    nc.scalar.copy(
                        conv_sb[:F, h2 * 512:(h2 + 1) * 512], ps)
                nc.vector.memset(conv_sb[F:F + 1, :], 1.0)
                conv_tiles.append(conv_sb)

            # ---- AllGather h.T (fires as soon as h_in lands)
            h_gat = dpool.tile([NCORES, HL, B], BF16)
            nc.gpsimd.collective_compute(
                "AllGather",
                mybir.AluOpType.bypass,
                replica_groups=[list(range(NCORES))],
                ins=[h_in[:].opt()],
                outs=[h_gat[:].opt()],
            )

            # remaining streams on gpsimd AFTER the collective: their slot
            # stalls resolve through sync/PE/DVE work only (deadlock-safe)
            for b in range(6, BL):
                pt = ppool.tile([128, NC_S, A], BF16, tag="proc")
                nc.gpsimd.dma_start(
                    pt[:], proc[b].rearrange("(p r) a -> p r a", r=NC_S))
                proc_tiles.append(pt)

            enc_tiles = []
            for b in range(10):
                et = epool.tile([128, ENC_T, E], BF16, tag="enc")
                nc.gpsimd.dma_start(
                    et[:], enc[b].rearrange("(p r) e -> p r e", r=NC_S))
                enc_tiles.append(et)


            for b in range(10, BL):
                et = epool.tile([128, ENC_T, E], BF16, tag="enc")
                nc.gpsimd.dma_start(
                    et[:], enc[b].rearrange("(p r) e -> p r e", r=NC_S))
                enc_tiles.append(et)

            hfull = cpool.tile([128, NKH, B], BF16)
            nc.scalar.dma_start(hfull[:], h_gat[:].rearrange("c p b -> p c b"))

            # ---- qry2 (full batch) + batch selection
            ps_q = mm_ps((B, A))
            for k in range(NKH):
                nc.tensor.matmul(ps_q, hfull[:, k, :], wqT[:, k, :],
                                 start=(k == 0), stop=False)
            nc.tensor.matmul(ps_q, ones_row[:], const_row[:],
                             start=False, stop=True)
            qry2_all = cpool.tile([B, A], F32)
            nc.vector.tensor_copy(qry2_all[:], ps_q)
            ps_q2 = mm_ps((BL, A))
            nc.tensor.matmul(ps_q2, sel_sb[:], qry2_all[:],
                             start=True, stop=True)
            qry2 = cpool.tile([BL, A], BF16)
            nc.vector.tensor_copy(qry2[:], ps_q2)

            # rhs_all[:, b, :] = [W_loc.T ; qry2[b]]  (K=33 fused loc+qry mm)
            rhs_all = cpool.tile([F + 1, BL, A], BF16)
            for b in range(BL):
                nc.vector.tensor_copy(rhs_all[:F, b, :], wlocT[:])
            qdram = dpool.tile([BL, A], BF16)
            nc.scalar.dma_start(qdram[:], qry2[:])
            qsrc = bass.AP(
                tensor=qdram[:].tensor,
                offset=qdram[:].offset,
                ap=[[BL * A, 1], [A, BL], [1, A]],
            )
            nc.scalar.dma_start(rhs_all[F:F + 1, :, :], qsrc)

            # ---- fused tail: scores -> group softmax -> context, streaming
            scoresT = cpool.tile([128, NC_S, BL], F32)
            wTt = cpool.tile([128, NC_S, BL], BF16)
            for g in range(BL // GRP):
                bs = range(g * GRP, (g + 1) * GRP)
                for b in bs:
                    conv_sb = conv_tiles[b]
                    ps_v = psV.tile([128, NC_S * A], F32, tag="v")
                    for c in range(NC_S):
                        nc.tensor.matmul(
                            ps_v[:, c * A:(c + 1) * A],
                            conv_sb[:, c:S:NC_S],
                            rhs_all[:, b, :],
                            start=True, stop=True)
                    v_sb = vpool.tile([128, NC_S, A], BF16, tag="v_sb")
                    nc.vector.tensor_add(
                        v_sb[:],
                        ps_v[:].rearrange("p (c a) -> p c a", c=NC_S),
                        proc_tiles[b][:])
                    nc.scalar.activation(v_sb[:], v_sb[:], AF.Tanh)
                    nc.vector.tensor_mul(v_sb[:], v_sb[:], wo_rep8[:])
                    nc.vector.reduce_sum(scoresT[:, :, b], v_sb[:],
                                         axis=mybir.AxisListType.X)

                # group softmax over S in [b, s] layout
                sc = wpool.tile([GRP, S], F32, tag="scg")
                for c in range(NC_S):
                    pe_t(sc[:, c * 128:(c + 1) * 128],
                         scoresT[:, c, g * GRP:(g + 1) * GRP], 128,
                         engine="act")
                mx = wpool.tile([GRP, 1], F32, tag="mxg")
                nc.vector.reduce_max(mx[:], sc[:], axis=mybir.AxisListType.X)
                nc.vector.tensor_scalar_mul(mx[:], mx[:], -1.0)
                sums = wpool.tile([GRP, 1], F32, tag="smg")
                nc.scalar.activation(sc[:], sc[:], AF.Exp, bias=mx[:],
                                     accum_out=sums[:])
                rs = wpool.tile([GRP, 1], F32, tag="rsg")
                nc.vector.reciprocal(rs[:], sums[:])
                nc.vector.tensor_scalar_mul(sc[:], sc[:], rs[:])
                for c in range(NC_S):
                    pe_t(wTt[:, c, g * GRP:(g + 1) * GRP],
                         sc[:, c * 128:(c + 1) * 128], GRP, engine="act")

                # context for this group
                for b in bs:
                    ps_x = psX.tile([1, E], F32, tag="ctx")
                    for c in range(NC_S):
                        nc.tensor.matmul(ps_x, wTt[:, c, b:b + 1],
                                         enc_tiles[b][:, c, :],
                                         start=(c == 0), stop=(c == NC_S - 1))
                    ctx_row = wpool.tile([1, E], F32, tag="ctxrow")
                    nc.scalar.copy(ctx_row[:], ps_x)
                    nc.sync.dma_start(out[b:b + 1, :], ctx_row[:])

            epool_cm.__exit__(None, None, None)

    nc.compile()
    return nc


_NC_CACHE = None


def _get_nc():
    global _NC_CACHE
    if _NC_CACHE is None:
        _NC_CACHE = build()
    return _NC_CACHE


def shard_inputs(prenet, prev_context, att_h, att_c, prev_weights, cum_weights,
                 enc_seq, proc_mem, mask, W_ih, W_hh, b_ih, b_hh, conv_w,
                 conv_b, W_loc, b_loc, W_q, b_q, W_out, **_unused):
    f = np.ascontiguousarray
    w_ih4 = np.asarray(W_ih, np.float32).reshape(4, H, PE_DIM)
    w_hh4 = np.asarray(W_hh, np.float32).reshape(4, H, H)
    b_ih4 = np.asarray(b_ih, np.float32).reshape(4, H)
    b_hh4 = np.asarray(b_hh, np.float32).reshape(4, H)
    in_maps = []
    for j in range(NCORES):
        bj = slice(BL * j, BL * (j + 1))
        hj = slice(HL * j, HL * (j + 1))
        sel = np.zeros((B, BL), np.float32)
        sel[BL * j:BL * (j + 1), :] = np.eye(BL, dtype=np.float32)
        in_maps.append({
            "prenet": f(np.asarray(prenet, np.float32)),
            "prev_ctx": f(np.asarray(prev_context, np.float32)),
            "att_h": f(np.asarray(att_h, np.float32)),
            "att_c_sh": f(np.asarray(att_c, np.float32)[:, hj]),
            "w_ih_sh": f(w_ih4[:, hj]),
            "w_hh_sh": f(w_hh4[:, hj]),
            "b_ih_sh": f(b_ih4[:, hj]),
            "b_hh_sh": f(b_hh4[:, hj]),
            "prev_w": f(np.asarray(prev_weights, np.float32)[bj]),
            "cum_w": f(np.asarray(cum_weights, np.float32)[bj]),
            "enc": f(np.asarray(enc_seq, np.float32)[bj]),
            "proc": f(np.asarray(proc_mem, np.float32)[bj]),
            "conv_w": f(np.asarray(conv_w, np.float32)),
            "conv_b": f(np.asarray(conv_b, np.float32).reshape(F, 1)),
            "w_loc": f(np.asarray(W_loc, np.float32)),
            "b_loc": f(np.asarray(b_loc, np.float32).reshape(1, A)),
            "w_q": f(np.asarray(W_q, np.float32)),
            "b_q": f(np.asarray(b_q, np.float32).reshape(1, A)),
            "w_out": f(np.asarray(W_out, np.float32).reshape(1, A)),
            "bsel": sel,
        })
    return in_maps


def kernel(**inputs):
    assert not np.any(np.asarray(inputs["mask"])), \
        "kernel assumes mask == 0 (softmax-shift support not implemented)"
    nc = _get_nc()
    in_maps = shard_inputs(**inputs)
    res = run_bass_kernel_spmd(nc, in_maps, core_ids=list(range(NCORES)))
    return np.concatenate([res.results[j]["out"] for j in range(NCORES)],
                          axis=0)


if __name__ == "__main__":
    rng = np.random.default_rng(0)
    print("building...")
    _get_nc()
    print("built ok")



# revision 26
# speedup vs baseline: 1.1343x; 1.1343x over previous
"""Trainium2 Bass kernel: Tacotron-style location-sensitive attention step.

Redesign vs v0 (297us):
  - ALL large tensors host-staged in bf16, pre-transposed/pre-arranged into
    the exact SBUF layouts the matmuls need -> zero on-device transposes,
    half the HBM traffic (25.6 MB/core vs 47 MB f32).
  - Big streams issued from t=0 on the two HWDGE queues (sync + scalar) in
    priority order: LSTM weights -> activations -> proc -> conv windows ->
    enc pairs.  No SWDGE for bulk data.
  - LSTM stays H-sharded (1/8 weights per core).  Each core computes a
    partial query qp = (W_q[:,hj] @ h[hj,:]) for the FULL batch; partials
    are summed across cores.  Exchange options:
      EXCHANGE="remote": 7x remote_dma_broadcast (SBUF->SBUF, 32KB each)
        -- latency ~ a few us, no ncfw barrier.  Sum of partials is
        permutation-invariant so logical/physical core mapping is irrelevant.
      EXCHANGE="ccl": ncfw AllGather via DRAM (slow but battle-tested).
  - Tail fully per-example pipelined: conv -> scores (qry folded in via a
    K=1 accumulate matmul) -> softmax (transpose-free: exp + accum sums,
    cross-partition sum via tiny matmul, 1/sum folded into the weights;
    max-subtraction skipped since |scores| <= sum|W_out| ~ 5) -> context
    matmul streamed against the enc tiles as they land.

kernel(**inputs) takes FULL numpy inputs and returns the FULL [128, 512]
float32 context.
"""

import sys

sys.path.insert(0, "/opt/trn_rl_repo")

import numpy as np
import ml_dtypes

import concourse.bass as bass
import concourse.mybir as mybir
from concourse import bacc
from concourse.bass_utils import run_bass_kernel_spmd
from concourse.bass import _add_dep_helper
from concourse.tile import TileContext

F32 = mybir.dt.float32
BF16 = mybir.dt.bfloat16
AF = mybir.ActivationFunctionType
NPBF = ml_dtypes.bfloat16

B, S, E, P, H, A, F, KW = 128, 1024, 512, 256, 1024, 128, 32, 31
NCORES = 8
BL = B // NCORES        # 16 examples per core
HL = H // NCORES        # 128 h rows per core
PE_DIM = P + E          # 768
NKI = PE_DIM // 128     # 6
NKH = H // 128          # 8
NK = NKI + NKH          # 14
NC_S = S // 128         # 8 s-chunks
TAPS = 2 * KW           # 62
ENC_BUFS = 5            # enc pair tiles in flight (2 examples / 2 MB each)
CONV_BUFS = 8
WIN_CH = 4              # examples per conv-window DMA chunk

EXCHANGE = "ccl"        # "remote" | "ccl"
DEBUG_TAPS = False      # extra DRAM outputs for stage-by-stage debugging


def build():
    nc = bacc.Bacc("TRN2", target_bir_lowering=False, debug=False,
                   num_devices=NCORES)

    dp = nc.declare_dram_parameter
    wt_d = dp("wt", [128, 4, NK, HL], BF16, isOutput=False)
    bias_d = dp("bias", [HL, 4], F32, isOutput=False)
    xt_d = dp("xt", [128, NKI, B], BF16, isOutput=False)
    aht_d = dp("aht", [128, NKH, B], BF16, isOutput=False)
    act_d = dp("act", [HL, B], BF16, isOutput=False)
    wqt_d = dp("wqt", [HL, A], BF16, isOutput=False)
    sel_d = dp("sel", [B, BL], BF16, isOutput=False)
    cst_d = dp("cst", [1, A], BF16, isOutput=False)
    wlt_d = dp("wlt", [F, A], BF16, isOutput=False)
    w2_d = dp("w2", [TAPS, F], BF16, isOutput=False)
    wo_d = dp("wo", [1, A], BF16, isOutput=False)
    win_d = dp("win", [TAPS, BL, S], BF16, isOutput=False)
    proc_d = dp("proc", [128, BL, NC_S, A], BF16, isOutput=False)
    enc_d = dp("enc", [128, BL, NC_S, E], BF16, isOutput=False)
    out_d = dp("out", [2, (BL // 2) * E], F32, isOutput=True)
    if DEBUG_TAPS:
        dbg_qsum_d = dp("dbg_qsum", [B, A], F32, isOutput=True)
        dbg_qrow_d = dp("dbg_qrow", [1, BL * A], F32, isOutput=True)
        dbg_conv_d = dp("dbg_conv", [F, S], F32, isOutput=True)
        dbg_sc_d = dp("dbg_sc", [128, BL * NC_S], F32, isOutput=True)
        dbg_wtt_d = dp("dbg_wtt", [128, BL * NC_S], F32, isOutput=True)
        dbg_ps_d = dp("dbg_ps", [128, BL], F32, isOutput=True)
        dbg_rsc_d = dp("dbg_rsc", [128, BL], F32, isOutput=True)

    with TileContext(nc) as tc:
        with (
            tc.tile_pool(name="const", bufs=1) as cpool,
            tc.tile_pool(name="win", bufs=2) as xpool,
            tc.tile_pool(name="conv", bufs=CONV_BUFS) as convpool,
            tc.tile_pool(name="vsb", bufs=2) as vpool,
            tc.tile_pool(name="enc", bufs=ENC_BUFS) as epool,
            tc.tile_pool(name="psA", bufs=2, space="PSUM") as psA,
            tc.tile_pool(name="psV", bufs=2, space="PSUM") as psV,
            tc.tile_pool(name="psX", bufs=2, space="PSUM") as psX,
            tc.tile_pool(name="dram", bufs=1, space="DRAM") as dpool,
        ):
            def mmA(rows, cols):
                t = psA.tile([128, 512], F32, tag="mm")
                return t[:rows, :cols]

            def mmX(rows, cols):
                t = psX.tile([128, 512], F32, tag="x")
                return t[:rows, :cols]

            # ---------------- priority DMAs (HWDGE, both queues) ----------
            # sync queue: wT half, LSTM activations, proc half, win, enc evens
            # scalar queue: wT half, small consts, proc half, win, enc odds
            wT = cpool.tile([128, 4, NK, HL], BF16)
            nc.sync.dma_start(wT[:, 0:2], wt_d[:, 0:2])
            nc.scalar.dma_start(wT[:, 2:4], wt_d[:, 2:4])

            xT = cpool.tile([128, NKI, B], BF16)
            nc.sync.dma_start(xT[:], xt_d[:])
            ahT = cpool.tile([128, NKH, B], BF16)
            nc.sync.dma_start(ahT[:], aht_d[:])
            acT = cpool.tile([HL, B], BF16)
            nc.sync.dma_start(acT[:], act_d[:])
            bias_sb = cpool.tile([HL, 4], F32)
            nc.sync.dma_start(bias_sb[:], bias_d[:])
            wqT = cpool.tile([HL, A], BF16)
            nc.sync.dma_start(wqT[:], wqt_d[:])
            sel_sb = cpool.tile([B, BL], BF16)
            nc.sync.dma_start(sel_sb[:], sel_d[:])

            cst_sb = cpool.tile([1, A], BF16)
            nc.scalar.dma_start(cst_sb[:], cst_d[:])
            wlocT = cpool.tile([F, A], BF16)
            nc.scalar.dma_start(wlocT[:], wlt_d[:])
            w2 = cpool.tile([TAPS, F], BF16)
            nc.scalar.dma_start(w2[:], w2_d[:])
            wo_row = cpool.tile([1, A], BF16)
            nc.scalar.dma_start(wo_row[:], wo_d[:])

            proc_sb = cpool.tile([128, BL, NC_S, A], BF16)
            nc.sync.dma_start(proc_sb[:, 0:8], proc_d[:, 0:8])
            nc.scalar.dma_start(proc_sb[:, 8:16], proc_d[:, 8:16])

            win_tiles = []
            for i in range(BL // WIN_CH):
                wt_t = xpool.tile([TAPS, WIN_CH, S], BF16, tag="win")
                eng = nc.sync if i % 2 == 0 else nc.scalar
                eng.dma_start(wt_t[:], win_d[:, i * WIN_CH:(i + 1) * WIN_CH])
                win_tiles.append(wt_t)

            enc_tiles = []
            for i in range(BL // 2):
                et = epool.tile([128, 2, NC_S, E], BF16, tag="enc")
                eng = nc.sync if i % 2 == 0 else nc.scalar
                eng.dma_start(et[:], enc_d[:, 2 * i:2 * i + 2])
                enc_tiles.append(et)

            # ---------------- constants ----------------------------------
            ones_row_f = cpool.tile([1, 128], F32)
            nc.vector.memset(ones_row_f[:], 1.0)
            ones_col_f = cpool.tile([128, 1], F32)
            nc.vector.memset(ones_col_f[:], 1.0)
            ones_row_b = cpool.tile([1, 128], BF16)
            nc.vector.memset(ones_row_b[:], 1.0)

            # wo replicated across partitions and s-chunks
            ps = mmA(128, A)
            nc.tensor.matmul(ps, ones_row_b[:], wo_row[:], start=True,
                             stop=True)
            wo_rep8 = cpool.tile([128, NC_S, A], BF16)
            for c in range(NC_S):
                eng = nc.scalar if c % 2 else nc.vector
                if c % 2:
                    nc.scalar.copy(wo_rep8[:, c, :], ps)
                else:
                    nc.vector.tensor_copy(wo_rep8[:, c, :], ps)

            # ---------------- LSTM gates (H-shard, full batch) ------------
            gate_sb = []
            for g in range(4):
                ps = mmA(HL, B)
                for k in range(NK):
                    rhs = xT[:, k, :] if k < NKI else ahT[:, k - NKI, :]
                    nc.tensor.matmul(ps, wT[:, g, k, :], rhs,
                                     start=(k == 0), stop=(k == NK - 1))
                sb = cpool.tile([HL, B], BF16, tag=f"gate{g}")
                fn = AF.Tanh if g == 2 else AF.Sigmoid
                nc.scalar.activation(sb[:], ps, fn, bias=bias_sb[:, g:g + 1])
                gate_sb.append(sb)

            cT = cpool.tile([HL, B], BF16)
            nc.vector.tensor_mul(cT[:], gate_sb[1][:], acT[:])
            tg = cpool.tile([HL, B], BF16)
            nc.vector.tensor_mul(tg[:], gate_sb[0][:], gate_sb[2][:])
            nc.vector.tensor_add(cT[:], cT[:], tg[:])
            nc.scalar.activation(tg[:], cT[:], AF.Tanh)
            hT = cpool.tile([HL, B], BF16)
            nc.vector.tensor_mul(hT[:], gate_sb[3][:], tg[:])

            # partial query for the FULL batch: qp[b, a]
            ps_q = mmA(B, A)
            nc.tensor.matmul(ps_q, hT[:], wqT[:], start=True, stop=True)
            qp_sb = cpool.tile([B, A], BF16)
            nc.vector.tensor_copy(qp_sb[:], ps_q)

            # ---------------- cross-core exchange of qp -------------------
            gather = cpool.tile([B, NCORES, A], BF16)
            qsum = cpool.tile([B, A], BF16)
            deferred_wait = None   # (instruction, sem, value) set post-exit
            if EXCHANGE == "remote":
                qsem = nc.alloc_semaphore("qx_remote")
                lsem = nc.alloc_semaphore("qx_local")
                for k in range(1, NCORES):
                    rd = [None] * NCORES
                    rd[k] = (0, k)
                    nc.gpsimd.remote_dma_broadcast(
                        gather[:, k, :], qp_sb[:],
                        remote_sem=qsem, local_sem=lsem, rdests=rd)
                trig = nc.gpsimd.trigger_dma(count=None)
                # Peers' qsem increments are invisible to the single-core
                # scheduler sim -> attach the actual HW wait after the Tile
                # context exits (see below).  The adds go on the gpsimd
                # engine so the HW-side stall blocks no other engine.
                add0 = nc.gpsimd.tensor_add(qsum[:], qp_sb[:],
                                            gather[:, 1, :])
                _add_dep_helper(add0.ins, trig.ins, sync=True,
                                reason="consume gather only after trigger")
                deferred_wait = (add0, qsem, 2 * (NCORES - 1))
                for k in range(2, NCORES):
                    nc.gpsimd.tensor_add(qsum[:], qsum[:],
                                         gather[:, k, :])
            else:
                qp_dram = dpool.tile([B, A], BF16)
                nc.sync.dma_start(qp_dram[:], qp_sb[:])
                gat_dram = dpool.tile([NCORES, B, A], BF16)
                nc.gpsimd.collective_compute(
                    "AllGather",
                    mybir.AluOpType.bypass,
                    replica_groups=[list(range(NCORES))],
                    ins=[qp_dram[:].opt()],
                    outs=[gat_dram[:].opt()],
                )
                nc.gpsimd.dma_start(
                    gather[:], gat_dram[:].rearrange("c b a -> b c a"))
                nc.vector.tensor_add(qsum[:], gather[:, 0, :],
                                     gather[:, 1, :])
                for k in range(2, NCORES):
                    nc.vector.tensor_add(qsum[:], qsum[:], gather[:, k, :])

            if DEBUG_TAPS:
                t = cpool.tile([B, A], F32, tag="dbgq")
                nc.vector.tensor_copy(t[:], qsum[:])
                nc.sync.dma_start(dbg_qsum_d[:], t[:])

            # select this core's 16 examples, add folded consts
            ps_s = mmA(BL, A)
            nc.tensor.matmul(ps_s, sel_sb[:], qsum[:], start=True, stop=False)
            nc.tensor.matmul(ps_s, ones_row_b[:, :BL], cst_sb[:],
                             start=False, stop=True)
            qry2 = cpool.tile([BL, A], BF16)
            nc.vector.tensor_copy(qry2[:], ps_s)
            # PE operands must sit at base partition 0/32/64 -> move the 16
            # query rows onto partition 0 (free dim) with a tiny SBUF->SBUF
            # DMA on the (otherwise idle) gpsimd queue.
            qrow = cpool.tile([1, BL, A], BF16)
            nc.gpsimd.dma_start(qrow[:], qry2[:])
            if DEBUG_TAPS:
                t = cpool.tile([1, BL * A], F32, tag="dbgqr")
                nc.vector.tensor_copy(
                    t[:], qrow[:].rearrange("p b a -> p (b a)"))
                nc.sync.dma_start(dbg_qrow_d[:], t[:])

            # ---------------- per-example tail ----------------------------
            sc_f = cpool.tile([128, BL, NC_S], F32)     # scores
            wTt = cpool.tile([128, BL, NC_S], BF16)     # softmax weights
            psums = cpool.tile([128, BL], F32)
            rcp = cpool.tile([1, BL], F32)
            rsc = cpool.tile([128, BL], F32)

            conv_tiles = []
            for b in range(BL):
                wt_t = win_tiles[b // WIN_CH]
                bi = b % WIN_CH
                conv_sb = convpool.tile([F, S], BF16, tag="conv")
                for h2 in range(2):
                    ps_c = mmA(F, 512)
                    nc.tensor.matmul(ps_c, w2[:],
                                     wt_t[:, bi, h2 * 512:(h2 + 1) * 512],
                                     start=True, stop=True)
                    if h2 == 0:
                        nc.vector.tensor_copy(
                            conv_sb[:, h2 * 512:(h2 + 1) * 512], ps_c)
                    else:
                        nc.scalar.copy(
                            conv_sb[:, h2 * 512:(h2 + 1) * 512], ps_c)
                conv_tiles.append(conv_sb)
                if DEBUG_TAPS and b == 0:
                    t = cpool.tile([F, S], F32, tag="dbgc")
                    nc.vector.tensor_copy(t[:], conv_sb[:])
                    nc.sync.dma_start(dbg_conv_d[:], t[:])

            out_sb = None
            for b in range(BL):
                if b % 8 == 0:
                    out_sb = cpool.tile([1, 8, E], F32, tag="out")
                conv_sb = conv_tiles[b]
                # scores: v = conv.Wloc + qry (K=1 accumulate) ; then
                # tanh(v + proc) . wo ; reduce over a
                ps_v = psV.tile([128, NC_S * A], F32, tag="v")
                for c in range(NC_S):
                    nc.tensor.matmul(ps_v[:, c * A:(c + 1) * A],
                                     conv_sb[:, c:S:NC_S], wlocT[:],
                                     start=True, stop=False)
                    nc.tensor.matmul(ps_v[:, c * A:(c + 1) * A],
                                     ones_row_b[:, :128], qrow[0:1, b, :],
                                     start=False, stop=True)
                v_sb = vpool.tile([128, NC_S, A], BF16, tag="v_sb")
                nc.vector.tensor_add(
                    v_sb[:],
                    ps_v[:].rearrange("p (c a) -> p c a", c=NC_S),
                    proc_sb[:, b])
                nc.scalar.activation(v_sb[:], v_sb[:], AF.Tanh)
                nc.vector.tensor_mul(v_sb[:], v_sb[:], wo_rep8[:])
                nc.vector.reduce_sum(sc_f[:, b, :], v_sb[:],
                                     axis=mybir.AxisListType.X)

                # softmax (no max-subtraction; 1/sum folded into weights)
                nc.scalar.activation(wTt[:, b, :], sc_f[:, b, :], AF.Exp,
                                     accum_out=psums[:, b:b + 1])
                ps_m = mmX(1, 1)
                nc.tensor.matmul(ps_m, psums[:, b:b + 1], ones_col_f[:],
                                 start=True, stop=True)
                nc.vector.reciprocal(rcp[:, b:b + 1], ps_m)
                ps_r = mmX(128, 1)
                nc.tensor.matmul(ps_r, ones_row_f[:], rcp[:, b:b + 1],
                                 start=True, stop=True)
                nc.vector.tensor_copy(rsc[:, b:b + 1], ps_r)
                nc.vector.tensor_scalar_mul(wTt[:, b, :], wTt[:, b, :],
                                            rsc[:, b:b + 1])

                # context
                et = enc_tiles[b // 2]
                ps_x = mmX(1, E)
                for c in range(NC_S):
                    nc.tensor.matmul(ps_x, wTt[:, b, c:c + 1],
                                     et[:, b % 2, c, :],
                                     start=(c == 0), stop=(c == NC_S - 1))
                if b % 2:
                    nc.scalar.copy(out_sb[:, b % 8, :], ps_x)
                else:
                    nc.vector.tensor_copy(out_sb[:, b % 8, :], ps_x)
                if b % 8 == 7:
                    nc.sync.dma_start(
                        out_d[b // 8:b // 8 + 1, :],
                        out_sb[:].rearrange("p b e -> p (b e)"))

            if DEBUG_TAPS:
                nc.sync.dma_start(
                    dbg_sc_d[:], sc_f[:].rearrange("p b c -> p (b c)"))
                tw = cpool.tile([128, BL, NC_S], F32, tag="dbgw")
                nc.vector.tensor_copy(tw[:], wTt[:])
                nc.sync.dma_start(
                    dbg_wtt_d[:], tw[:].rearrange("p b c -> p (b c)"))
                nc.sync.dma_start(dbg_ps_d[:], psums[:])
                nc.sync.dma_start(dbg_rsc_d[:], rsc[:])

    if deferred_wait is not None:
        # Post-scheduling: bake the remote-sem wait into the first consumer
        # of the gathered slots.  The scheduler never simulates it (it would
        # deadlock -- the increments come from peer cores); the NEFF gets it.
        ins, sem, val = deferred_wait
        ins.wait_op(sem, val, "sem-ge", check=False)

    nc.compile()
    return nc


_NC_CACHE = None


def _get_nc():
    global _NC_CACHE
    if _NC_CACHE is None:
        _NC_CACHE = build()
    return _NC_CACHE


def shard_inputs(prenet, prev_context, att_h, att_c, prev_weights, cum_weights,
                 enc_seq, proc_mem, mask, W_ih, W_hh, b_ih, b_hh, conv_w,
                 conv_b, W_loc, b_loc, W_q, b_q, W_out, **_unused):
    f32 = np.float32
    c = np.ascontiguousarray

    def bf(x):
        return c(np.asarray(x).astype(NPBF))

    W4 = np.concatenate([np.asarray(W_ih, f32).reshape(4, H, PE_DIM),
                         np.asarray(W_hh, f32).reshape(4, H, H)], axis=2)
    bias4 = (np.asarray(b_ih, f32) + np.asarray(b_hh, f32)).reshape(4, H)

    x_full = np.concatenate([np.asarray(prenet, f32),
                             np.asarray(prev_context, f32)], axis=1)  # [B,768]
    xt_host = bf(x_full.T.reshape(NKI, 128, B).transpose(1, 0, 2))
    aht_host = bf(np.asarray(att_h, f32).T.reshape(NKH, 128, B)
                  .transpose(1, 0, 2))

    cst_host = bf((np.asarray(W_loc, f32) @ np.asarray(conv_b, f32)
                   + np.asarray(b_loc, f32)
                   + np.asarray(b_q, f32)).reshape(1, A))
    wlt_host = bf(np.asarray(W_loc, f32).T)                     # [F, A]
    w2_host = bf(np.asarray(conv_w, f32).transpose(1, 2, 0).reshape(TAPS, F))
    wo_host = bf(np.asarray(W_out, f32).reshape(1, A))

    cum = np.asarray(cum_weights, f32)
    prev = np.asarray(prev_weights, f32)
    att_c = np.asarray(att_c, f32)
    W_q = np.asarray(W_q, f32)
    enc_seq = np.asarray(enc_seq, f32)
    proc_mem = np.asarray(proc_mem, f32)

    in_maps = []
    for j in range(NCORES):
        bj = slice(BL * j, BL * (j + 1))
        hj = slice(HL * j, HL * (j + 1))

        W4s = W4[:, hj, :]                                      # [4,128,1792]
        wt_host = bf(W4s.reshape(4, HL, NK, 128).transpose(3, 0, 2, 1))

        sel = np.zeros((B, BL), f32)
        sel[bj, :] = np.eye(BL, dtype=f32)

        xp = np.zeros((BL, 2, S + KW - 1), f32)
        xp[:, 0, KW // 2:KW // 2 + S] = cum[bj]
        xp[:, 1, KW // 2:KW // 2 + S] = prev[bj]
        sw = np.lib.stride_tricks.sliding_window_view(xp, S, axis=2)
        win_host = bf(sw.transpose(1, 2, 0, 3).reshape(TAPS, BL, S))

        proc_host = bf(proc_mem[bj].reshape(BL, 128, NC_S, A)
                       .transpose(1, 0, 2, 3))
        enc_host = bf(enc_seq[bj].reshape(BL, 128, NC_S, E)
                      .transpose(1, 0, 2, 3))

        in_maps.append({
            "wt": wt_host,
            "bias": c(bias4[:, hj].T),
            "xt": xt_host,
            "aht": aht_host,
            "act": bf(att_c[:, hj].T),
            "wqt": bf(W_q[:, hj].T),
            "sel": bf(sel),
            "cst": cst_host,
            "wlt": wlt_host,
            "w2": w2_host,
            "wo": wo_host,
            "win": win_host,
            "proc": proc_host,
            "enc": enc_host,
        })
    return in_maps


def kernel(**inputs):
    assert not np.any(np.asarray(inputs["mask"])), \
        "kernel assumes mask == 0 (softmax-shift support not implemented)"
    nc = _get_nc()
    in_maps = shard_inputs(**inputs)
    res = run_bass_kernel_spmd(nc, in_maps, core_ids=list(range(NCORES)))
    return np.concatenate(
        [np.asarray(res.results[j]["out"]).reshape(BL, E)
         for j in range(NCORES)], axis=0)


if __name__ == "__main__":
    print("building...")
    _get_nc()
    print("built ok")
